# revision 29
# baseline (speedup 1.0000x reference)
"""Trainium2 Bass kernel for a pre-LN multi-head attention block.

Computes, for x of shape (4, 2048, 512):
    xn  = LayerNorm(x) * gamma + beta
    q/k/v = xn @ W{q,k,v}.T + b{q,k,v}          (8 heads, dk=64)
    attn  = softmax(q k^T / sqrt(dk)) @ v
    out   = attn @ Wo.T + bo
LN gamma/beta are folded into the projection weights host-side
(W' = W * gamma, b' = W @ beta + b), so on-chip LN is just (x-mu)*rstd.

Sharding: 8 cores = (4 batches) x (2 query-halves). Every core computes
LayerNorm + K/V for its batch's full 2048-token sequence and Q only for
its 1024 queries, so per-core outputs are disjoint row blocks of the
final result and the host gather is pure concatenation (no reduction).
The SPMD program is identical on all cores; per-core differences are
data-only (each core's x is passed with its query rows first -- attention
is invariant to key ordering as long as K and V share it).

On-chip layout (per core): everything is kept transposed,
scores^T[key, query], so the softmax reduction lands on the PE via a
ones-column appended to V (row 64 of the PV accumulator = softmax
denominators), and exp() is the only ScalarE pass over the n^2 scores.
Matmul operands are bf16; accumulation is fp32 in PSUM.

Schedule: the attention inner loop is ScalarE(exp)-paced at ~1.15us per
key tile, so every other engine must fit in exp's shadow. All Q/K/V
projection matmuls are interleaved into the LayerNorm loop (the PE is
drain-limited there), with PSUM drains spread across engines: xnT and
K-bias drains on ScalarE (Identity, same act table as Exp), V-bias on
GpSimd, Q-bias on DVE. The PE gets no filler work: TensorE runs at
2.4GHz only for a ~164us activity-triggered boost window before HAM
drops it to 1.2GHz, so PE work is hoarded for the attention span.
Softmax normalization is one reciprocal_approx_fast + one GpSimd
partition-broadcast + one wide multiply per head, deferred into the
next head's loop; the last head runs as two query-half passes so its
normalize + output projection overlaps the second half's attention.
"""

import ml_dtypes
import numpy as np

import concourse.bass as bass
import concourse.mybir as mybir
import concourse.tile as tile
from concourse import bacc
from concourse.bass_utils import run_bass_kernel_spmd
from concourse.masks import make_identity

F32 = mybir.dt.float32
BF16 = mybir.dt.bfloat16
ALU = mybir.AluOpType
ACTF = mybir.ActivationFunctionType

P = 128          # partitions
DIM = 512        # model dim
H = 8            # heads
DK = 64          # head dim
NTOK = 2048      # tokens per core (one batch's sequence)
NQ = 1024        # queries per core (half the sequence)
CC = DIM // P    # 4 contraction chunks of 128
TT = NTOK // P   # 16 token tiles
JT = NTOK // P   # 16 key tiles
NB = 512         # moving-operand limit per matmul
EPS = 1e-5
SCALE = DK ** -0.5

N_CORES = 8
_BUILT = None


def _build():
    nc = bacc.Bacc("TRN2", target_bir_lowering=False, debug=False,
                   num_devices=N_CORES)

    xq = nc.dram_tensor("xq", [NTOK, DIM], F32, kind="ExternalInput")
    wqT = nc.dram_tensor("wqT", [DIM, DIM], BF16, kind="ExternalInput")
    wkT = nc.dram_tensor("wkT", [DIM, DIM], BF16, kind="ExternalInput")
    wvT = nc.dram_tensor("wvT", [DIM, DIM], BF16, kind="ExternalInput")
    woT = nc.dram_tensor("woT", [DK, H, DIM], BF16, kind="ExternalInput")
    qb_c = nc.dram_tensor("qb_c", [P, CC], F32, kind="ExternalInput")
    kb_c = nc.dram_tensor("kb_c", [P, CC], F32, kind="ExternalInput")
    bv_t = nc.dram_tensor("bv_t", [DK, H], F32, kind="ExternalInput")
    bo_b = nc.dram_tensor("bo_b", [P, DIM], F32, kind="ExternalInput")
    y = nc.dram_tensor("y", [NQ, DIM], F32, kind="ExternalOutput")

    with tile.TileContext(nc) as tc:
        with (
            tc.tile_pool(name="const", bufs=1) as const,
            tc.tile_pool(name="persist", bufs=1) as persist,
            tc.tile_pool(name="lnp", bufs=6) as lnp,
            tc.tile_pool(name="stp", bufs=8) as stp,
            tc.tile_pool(name="epp", bufs=3) as epp,
            tc.tile_pool(name="otp", bufs=2) as otp,
            tc.tile_pool(name="rpp", bufs=4) as rpp,
            tc.tile_pool(name="outp", bufs=3) as outp,
            # PSUM: 4 banks (scores) + 2 banks (O accum) + 2 banks (work)
            tc.tile_pool(name="spp", bufs=2, space="PSUM") as spp,
            tc.tile_pool(name="opp", bufs=1, space="PSUM") as opp,
            tc.tile_pool(name="wpp", bufs=2, space="PSUM") as wpp,
        ):
            # x tile loads first -- LayerNorm is the head of the critical
            # path; weights are not needed until the projections.
            ident = const.tile([P, P], BF16)
            make_identity(nc, ident)
            identf = const.tile([P, P], F32)
            make_identity(nc, identf)
            onesp = const.tile([P, 1], F32)
            nc.vector.memset(onesp, 1.0)
            xts = []
            for tt in range(TT):
                xt = lnp.tile([P, DIM], F32, tag="xt", name=f"xt{tt}")
                nc.sync.dma_start(out=xt, in_=xq.ap()[tt * P:(tt + 1) * P, :])
                xts.append(xt)

            qb = const.tile([P, CC], F32)
            nc.sync.dma_start(out=qb, in_=qb_c.ap())
            kb = const.tile([P, CC], F32)
            nc.sync.dma_start(out=kb, in_=kb_c.ap())
            bvt = const.tile([DK, H], F32)
            nc.sync.dma_start(out=bvt, in_=bv_t.ap())
            bob = const.tile([P, DIM], F32)
            nc.sync.dma_start(out=bob, in_=bo_b.ap())
            epst = const.tile([P, 1], F32)
            nc.vector.memset(epst, EPS)
            wv = const.tile([P, CC, DIM], BF16)
            nc.sync.dma_start(out=wv, in_=wvT.ap().rearrange(
                "(cc p) d -> p cc d", p=P))
            wq = const.tile([P, CC, DIM], BF16)
            nc.sync.dma_start(out=wq, in_=wqT.ap().rearrange(
                "(cc p) d -> p cc d", p=P))
            wk = const.tile([P, CC, DIM], BF16)
            nc.sync.dma_start(out=wk, in_=wkT.ap().rearrange(
                "(cc p) d -> p cc d", p=P))
            wo = const.tile([DK, H, DIM], BF16)
            nc.sync.dma_start(out=wo, in_=woT.ap())

            # Persistent activations (alive across phases).
            xnT = persist.tile([P, CC, NTOK], BF16)    # xn^T; chunk cc = dims [128cc,..)
            qt = persist.tile([P, CC, NQ], BF16)       # Q^T; tile t = q-dims [128t,..)
            kt = persist.tile([P, CC, NTOK], BF16)     # K^T
            vp = persist.tile([P, JT, H, DK + 2], BF16)  # V' per key tile: [V_h | 1 | 0]
            onT = persist.tile([DK, H, NQ], BF16)      # normalized O^T per head

            nc.vector.memset(vp[:, :, :, DK], 1.0)
            nc.vector.memset(vp[:, :, :, DK + 1], 0.0)

            # Deferred-work queue: closures popped at fixed points so slack
            # engines absorb them without stalling the pacing engine.
            work_q = []

            def pump(n=1):
                for _ in range(n):
                    if work_q:
                        work_q.pop(0)()

            def qk_chunk(w, dst, t, ib, on_act, bias):
                # one (128, NB) psum chunk of the Q^T or K^T projection
                ps = wpp.tile([P, NB], F32, tag="w", name=f"qk{t}_{ib}_{w.name}")
                for cc in range(CC):
                    nc.tensor.matmul(ps, lhsT=w[:, cc, t * P:(t + 1) * P],
                                     rhs=xnT[:, cc, ib * NB:(ib + 1) * NB],
                                     start=(cc == 0), stop=(cc == CC - 1))
                dstv = dst[:, t, ib * NB:(ib + 1) * NB]
                if on_act:
                    nc.scalar.activation(out=dstv, in_=ps, func=ACTF.Identity,
                                         bias=bias[:, t:t + 1])
                else:
                    nc.vector.tensor_scalar(
                        out=dstv, in0=ps, scalar1=bias[:, t:t + 1],
                        scalar2=None, op0=ALU.add)

            def v_proj(j):
                # V bias is NOT added here: softmax weights sum to 1, so
                # (V+bv) attention == V attention + bv, applied per-head in
                # norm3 where it is a cheap per-partition scalar add.
                ps = wpp.tile([P, DIM], F32, tag="w", name=f"v{j}")
                for cc in range(CC):
                    nc.tensor.matmul(ps, lhsT=xnT[:, cc, j * P:(j + 1) * P],
                                     rhs=wv[:, cc, :],
                                     start=(cc == 0), stop=(cc == CC - 1))
                nc.scalar.activation(
                    out=vp[:, j, :, 0:DK],
                    in_=ps.rearrange("p (h d) -> p h d", d=DK),
                    func=ACTF.Copy)

            # ---- LayerNorm + transpose with ALL of V/Q/K interleaved in.
            # Q/K chunks for token block ib become ready once tiles
            # 4ib..4ib+3 are transposed; two run after each later tile.
            for tt in range(TT):
                xt = xts[tt]
                stats = stp.tile([P, 6], F32)
                nc.vector.bn_stats(out=stats, in_=xt)
                mv = stp.tile([P, 2], F32)
                nc.vector.bn_aggr(out=mv, in_=stats)
                rstd = stp.tile([P, 1], F32)
                nc.scalar.activation(out=rstd, in_=mv[:, 1:2], func=ACTF.Sqrt,
                                     bias=epst)
                nc.vector.reciprocal(out=rstd, in_=rstd)
                nmr = stp.tile([P, 1], F32)
                nc.vector.tensor_scalar(out=nmr, in0=mv[:, 0:1], scalar1=rstd,
                                        scalar2=-1.0, op0=ALU.mult,
                                        op1=ALU.mult)
                z = lnp.tile([P, DIM], BF16, tag="z")
                nc.scalar.activation(out=z, in_=xt, func=ACTF.Identity,
                                     scale=rstd, bias=nmr)
                for cc in range(CC):
                    ztp = spp.tile([P, P], BF16, tag="sps", name=f"t{tt}_{cc}")
                    nc.tensor.transpose(ztp, z[:, cc * P:(cc + 1) * P], ident)
                    nc.vector.tensor_copy(
                        out=xnT[:, cc, tt * P:(tt + 1) * P], in_=ztp)
                v_proj(tt)
                if tt % 4 == 3:
                    ib = tt // 4
                    for t in range(CC):
                        if ib < NQ // NB:
                            work_q.append(
                                lambda t=t, ib=ib: qk_chunk(wq, qt, t, ib,
                                                            False, qb))
                        work_q.append(
                            lambda t=t, ib=ib: qk_chunk(wk, kt, t, ib,
                                                        True, kb))
                if tt >= 3:
                    pump(2)

            def attention(h, cols, accs):
                # One pass of head h over query columns `cols` (start, n).
                # accs: (ops, ot) PSUM accumulator + SBUF drain, shared
                # between the two half passes of the split last head.
                hp, hm = divmod(h, 2)
                kt_h = kt[hm * DK:(hm + 1) * DK, hp, :]
                qt_h = qt[hm * DK:(hm + 1) * DK, hp, :]
                c0, cn = cols
                ops, ot = accs
                for j in range(JT):
                    sps = spp.tile([P, cn], F32, tag="sps", name=f"s{h}{j}{c0}")
                    for ib in range(cn // NB):
                        nc.tensor.matmul(
                            sps[:, ib * NB:(ib + 1) * NB],
                            lhsT=kt_h[:, j * P:(j + 1) * P],
                            rhs=qt_h[:, c0 + ib * NB:c0 + (ib + 1) * NB],
                            start=True, stop=True)
                    et = epp.tile([P, cn], BF16, tag="et", name=f"e{h}{j}{c0}")
                    nc.scalar.activation(out=et, in_=sps, func=ACTF.Exp,
                                         scale=SCALE)
                    for ib in range(cn // NB):
                        nc.tensor.matmul(
                            ops[:, c0 + ib * NB:c0 + (ib + 1) * NB],
                            lhsT=vp[:, j, h, :],
                            rhs=et[:, ib * NB:(ib + 1) * NB],
                            start=(j == 0), stop=(j == JT - 1))
                    if j % 3 == 2:
                        pump(1)
                # Drain the PSUM accumulator eagerly -- the next head's PV
                # start=True reuses this bank and waits on this copy.
                cs = slice(c0, c0 + cn)
                nc.vector.tensor_copy(out=ot[:, cs], in_=ops[0:DK + 1, cs])

                # Softmax denominators live on one partition line ([1, cn]),
                # where a reciprocal runs ~7 cycles/elem on a single DVE
                # lane. Transpose them onto partitions with the PE, take one
                # wide cheap reciprocal, transpose back, then broadcast.
                # Split into 3 pieces so each PE insert hides in exp's
                # shadow and cross-engine waits land between pumps.
                nck = cn // P
                rinvT = rpp.tile([P, nck], F32, tag="r", name=f"r{h}{c0}")
                rr = rpp.tile([1, cn], F32, tag="rr", name=f"rr{h}{c0}")
                rb = rpp.tile([DK, cn], F32, tag="rb", name=f"rb{h}{c0}")

                def norm1(h=h, ot=ot, c0=c0, nck=nck, rinvT=rinvT):
                    dnT = wpp.tile([P, nck], F32, tag="w", name=f"dn{h}{c0}")
                    for r in range(nck):
                        nc.tensor.transpose(
                            dnT[:, r:r + 1],
                            ot[DK:DK + 1, c0 + r * P:c0 + (r + 1) * P],
                            onesp[DK:DK + 1, :])
                    nc.vector.reciprocal(out=rinvT, in_=dnT)

                def norm2(h=h, c0=c0, nck=nck, rinvT=rinvT, rr=rr):
                    # transpose back to partition 0 (GPSIMD broadcast may
                    # only read partition-0-based APs), in bank-sized halves
                    for q in range(0, nck, CC):
                        qn = min(CC, nck - q)
                        rrp = wpp.tile([1, CC * P], F32, tag="w",
                                       name=f"rr{h}{c0}{q}")
                        for r in range(qn):
                            nc.tensor.transpose(
                                rrp[:, r * P:(r + 1) * P],
                                rinvT[:, q + r:q + r + 1], identf)
                        nc.vector.tensor_copy(
                            out=rr[:, q * P:(q + qn) * P],
                            in_=rrp[:, 0:qn * P])

                def norm3(h=h, ot=ot, cs=cs, rr=rr, rb=rb):
                    nc.gpsimd.partition_broadcast(rb, rr, channels=DK)
                    nc.vector.tensor_tensor(out=onT[:, h, cs],
                                            in0=ot[0:DK, cs], in1=rb,
                                            op=ALU.mult)
                    # deferred V bias (see v_proj)
                    nc.vector.tensor_scalar(
                        out=onT[:, h, cs], in0=onT[:, h, cs],
                        scalar1=bvt[:, h:h + 1], scalar2=None, op0=ALU.add)
                return [norm1, norm2, norm3]

            def y_proj(it):
                yps = wpp.tile([P, DIM], F32, tag="w", name=f"y{it}")
                for h in range(H):
                    nc.tensor.matmul(
                        yps, lhsT=onT[:, h, it * P:(it + 1) * P],
                        rhs=wo[:, h, :],
                        start=(h == 0), stop=(h == H - 1))
                yo = outp.tile([P, DIM], F32)
                nc.vector.tensor_tensor(out=yo, in0=yps, in1=bob, op=ALU.add)
                nc.sync.dma_start(out=y.ap()[it * P:(it + 1) * P, :], in_=yo)

            def head_accs(h):
                ops = opp.tile([DK + 2, NQ], F32, tag="ops", name=f"o{h}")
                ot = otp.tile([DK + 1, NQ], F32, tag="ot", name=f"ot{h}")
                return ops, ot

            for h in range(H - 1):
                accs = head_accs(h)
                work_q.extend(attention(h, (0, NQ), accs))
            # Last head in two query-half passes: half A's normalize and
            # y tiles overlap half B's attention; only half B's tail is
            # serial.
            accs = head_accs(H - 1)
            work_q.extend(attention(H - 1, (0, NQ // 2), accs))
            for it in range(NQ // (2 * P)):
                work_q.append(lambda it=it: y_proj(it))
            normb = attention(H - 1, (NQ // 2, NQ // 2), accs)
            pump(len(work_q))
            for nb in normb:
                nb()
            for it in range(NQ // (2 * P), NQ // P):
                y_proj(it)

    nc.compile()
    return nc


def _get_nc():
    global _BUILT
    if _BUILT is None:
        _BUILT = _build()
    return _BUILT


def prep_in_maps(inputs):
    x = np.asarray(inputs["x"], np.float32)
    B, N, D = x.shape
    assert (B, N, D) == (4, 2048, 512)

    gamma = np.asarray(inputs["ln_gamma"], np.float32)
    beta = np.asarray(inputs["ln_beta"], np.float32)

    def cols(v):  # (512,) -> (128, 4): column t = v[128t:128(t+1)]
        return np.ascontiguousarray(np.asarray(v, np.float32).reshape(CC, P).T)

    def bcast(v):  # (512,) -> (128, 512)
        return np.ascontiguousarray(
            np.broadcast_to(np.asarray(v, np.float32), (P, DIM)))

    def fold(W, b):
        # y = (z*gamma + beta) @ W.T + b  ==  z @ (gamma[:,None]*W.T) + (W@beta + b)
        W = np.asarray(W, np.float32)
        b = np.asarray(b, np.float32)
        return gamma[:, None] * W.T, W @ beta + b

    bf16 = ml_dtypes.bfloat16
    WqT, bq = fold(inputs["Wq"], inputs["bq"])
    WkT, bk = fold(inputs["Wk"], inputs["bk"])
    WvT, bv = fold(inputs["Wv"], inputs["bv"])
    common = {
        "wqT": np.ascontiguousarray(WqT.astype(bf16)),
        "wkT": np.ascontiguousarray(WkT.astype(bf16)),
        "wvT": np.ascontiguousarray(WvT.astype(bf16)),
        "woT": np.ascontiguousarray(
            np.asarray(inputs["Wo"], np.float32).T
            .reshape(H, DK, DIM).transpose(1, 0, 2).astype(bf16)),
        "qb_c": cols(bq), "kb_c": cols(bk),
        "bv_t": np.ascontiguousarray(bv.reshape(H, DK).T),
        "bo_b": bcast(inputs["bo"]),
    }
    in_maps = []
    for c in range(N_CORES):
        b, half = divmod(c, 2)
        o = half * NQ
        xc = np.concatenate([x[b, o:o + NQ], x[b, NQ - o:N - o]], axis=0)
        in_maps.append({"xq": np.ascontiguousarray(xc), **common})
    return in_maps


def kernel(x, ln_gamma, ln_beta, Wq, bq, Wk, bk, Wv, bv, Wo, bo):
    in_maps = prep_in_maps(dict(
        x=x, ln_gamma=ln_gamma, ln_beta=ln_beta, Wq=Wq, bq=bq, Wk=Wk, bk=bk,
        Wv=Wv, bv=bv, Wo=Wo, bo=bo))

    nc = _get_nc()
    res = run_bass_kernel_spmd(nc, in_maps, core_ids=list(range(N_CORES)))

    B, N, D = 4, 2048, DIM
    out = np.empty((B, N, D), np.float32)
    for c in range(N_CORES):
        b, half = divmod(c, 2)
        o = half * NQ
        out[b, o:o + NQ] = res.results[c]["y"]
    return out
